# revision 50
# baseline (speedup 1.0000x reference)
"""Self-contained Trainium2 Bass kernel for nn_MoEWithDeepEP (8 NeuronCores).

Expert-parallel MoE, DeepEP-style split:
  host   - exact fp32 router (sigmoid top-2 + normalize), dispatch planning,
           token packing (the all-to-all bookkeeping), weighted combine.
  device - ONE fused kernel per core: 8 local experts' SwiGLU GEMMs over
           exactly-counted token segments (tokens ride the matmul free dim,
           so there is no capacity padding in compute) + the shared expert
           over this core's 1/8 token shard.

All device tensors are pre-shuffled on the host into partition-major
[128, F] contiguous layouts so every DMA is a dense 2D copy (few, large
descriptors). DMA issue is spread over four engine queues. The PE is
warmed with throwaway matmuls during the input DMA lead-in so real work
runs at full clock.

The per-slot segment capacities depend on the routing realized by the
inputs, so the device program is built (and cached) per capacity tuple.
"""
import sys
for _p in ("/opt/trn_rl_repo", "/root/.axon_site/_ro/trn_rl_repo"):
    if _p not in sys.path:
        sys.path.insert(0, _p)

import numpy as np

N = 8192          # tokens
D = 512           # model dim
E = 64            # experts
K = 2             # top-k
H = 256           # expert hidden
HS = 512          # shared hidden (H * num_shared)
NCORES = 8
NSLOT = E // NCORES   # 8 expert slots per core
NS = N // NCORES      # shared-expert tokens per core
GRP = 256             # shared-expert token group
NGRP = NS // GRP
ROUTE_SCALE = 2.5
WARM_MM = 32          # PE warmup matmuls during DMA lead-in


def _mk_bacc():
    from concourse import bacc

    return bacc.Bacc(
        "TRN2",
        target_bir_lowering=False,
        debug=False,
        enable_asserts=False,
        num_devices=NCORES,
    )


# ---------------- host-side routing / planning ----------------

def route_and_plan(x, gate_w):
    """Exact fp32 router + expert->core assignment + slot capacities."""
    xf = np.ascontiguousarray(np.asarray(x, np.float32).reshape(N, D))
    logits = xf @ np.asarray(gate_w, np.float32).T
    scores = 1.0 / (1.0 + np.exp(-logits))
    top_idx = np.argsort(-scores, axis=1, kind="stable")[:, :K]
    tsc = np.take_along_axis(scores, top_idx, 1)
    gat = (tsc / (tsc.sum(1, keepdims=True) + 1e-20) * ROUTE_SCALE).astype(
        np.float32
    )
    counts = np.bincount(top_idx.ravel(), minlength=E)

    toks, gvals = [], []
    for e in range(E):
        t, kk = np.nonzero(top_idx == e)
        toks.append(t)
        gvals.append(gat[t, kk])

    # LPT: exactly NSLOT experts per core, balancing token load
    order = np.argsort(-counts, kind="stable")
    loads = [0] * NCORES
    assign = [[] for _ in range(NCORES)]
    for e in order:
        c = min(
            (c for c in range(NCORES) if len(assign[c]) < NSLOT),
            key=lambda c: loads[c],
        )
        assign[c].append(int(e))
        loads[c] += int(counts[e])
    for c in range(NCORES):
        assign[c].sort(key=lambda e: -counts[e])

    # slot capacities: rank-wise max across cores, 8-aligned
    caps = tuple(
        max(8, int(-(-max(counts[assign[c][j]] for c in range(NCORES)) // 8) * 8))
        for j in range(NSLOT)
    )
    return xf, toks, gvals, assign, caps


# ---------------- device kernel ----------------

def build_kernel(caps):
    import concourse.bass as bass
    import concourse.tile as tile
    from concourse import mybir

    dt = mybir.dt
    AF = mybir.ActivationFunctionType
    OP = mybir.AluOpType
    ts = bass.ts
    nc = _mk_bacc()

    starts = np.concatenate([[0], np.cumsum(caps)]).astype(int)
    P = int(starts[-1])

    # all inputs/outputs pre-shuffled to partition-major [128, F] layouts
    xd = nc.dram_tensor("xd", [128, 4 * P], dt.float16, kind="ExternalInput")
    w13s = nc.dram_tensor("w13s", [NSLOT, 128, 8 * H], dt.float16,
                          kind="ExternalInput")
    w2s = nc.dram_tensor("w2s", [NSLOT, 128, 2 * D], dt.float16,
                         kind="ExternalInput")
    sw13 = nc.dram_tensor("sw13", [128, 8 * HS], dt.float16, kind="ExternalInput")
    sw2p = nc.dram_tensor("sw2p", [128, 4 * D], dt.float16, kind="ExternalInput")
    xsp = nc.dram_tensor("xsp", [128, NGRP * 4 * GRP], dt.float16,
                         kind="ExternalInput")

    yT = nc.dram_tensor("yT", [128, 4 * P], dt.float16, kind="ExternalOutput")
    ysh = nc.dram_tensor("ysh", [128, NGRP * 2 * D], dt.float16,
                         kind="ExternalOutput")

    with tile.TileContext(nc) as tc:
        with (
            tc.tile_pool(name="const", bufs=1) as cpool,
            tc.tile_pool(name="ew", bufs=8) as ewpool,
            tc.tile_pool(name="xe", bufs=8) as xpool,
            tc.tile_pool(name="work", bufs=2) as wpool,
            tc.tile_pool(name="hps", bufs=4, space="PSUM") as hpsum,
            tc.tile_pool(name="yps", bufs=4, space="PSUM") as ypsum,
        ):
            # ---- PE warmup: throwaway matmuls with no input deps ----
            wsrc = cpool.tile([128, 128], dt.float16, name="wsrc")
            nc.vector.memset(wsrc[:], 0)
            for _ in range(WARM_MM):
                wps = ypsum.tile([128, 128], dt.float32, tag="py", name="wps")
                nc.tensor.matmul(
                    wps[:], lhsT=wsrc[:], rhs=wsrc[:, 0:1].to_broadcast([128, 128]),
                    start=True, stop=True, skip_group_check=True,
                )

            sw13_sb = cpool.tile([128, 8, HS], dt.float16, name="sw13_sb")
            sw2_sb = cpool.tile([128, 4, D], dt.float16, name="sw2_sb")
            xs_sb = cpool.tile([128, NGRP, 4, GRP], dt.float16, name="xs_sb")

            # ---- preload: pure-load queues, no store ever blocks a load.
            # sync carries weights, gpsimd carries activations; pool ring
            # back-pressure paces the prefetch depth automatically.
            w13_tiles, w2_tiles, xe_tiles = {}, {}, {}

            def load_w13(j):
                # hc-major halves: the first 0.26MB half unblocks L1 hc=0
                t = ewpool.tile([128, 2, 8, 128], dt.float16, tag="w13",
                                name="w13_sb")
                for hc in range(2):
                    nc.sync.dma_start(
                        t[:, hc],
                        w13s.ap()[j][:, ts(hc, 8 * 128)].rearrange(
                            "p (c h) -> p c h", c=8
                        ),
                    )
                w13_tiles[j] = t

            def load_w2(j):
                t = ewpool.tile([128, 2, D], dt.float16, tag="w2", name="w2_sb")
                nc.sync.dma_start(
                    t[:], w2s.ap()[j].rearrange("p (c d) -> p c d", c=2)
                )
                w2_tiles[j] = t

            def load_sw13():
                nc.gpsimd.dma_start(
                    sw13_sb[:], sw13.ap().rearrange("p (c h) -> p c h", c=8)
                )

            def load_sw2():
                nc.gpsimd.dma_start(
                    sw2_sb[:], sw2p.ap().rearrange("p (c d) -> p c d", c=4)
                )

            def load_xs(g):
                nc.gpsimd.dma_start(
                    xs_sb[:, g],
                    xsp.ap()[:, ts(g, 4 * GRP)].rearrange(
                        "p (c t) -> p c t", c=4
                    ),
                )

            def load_xe(j, eng=None):
                C = int(caps[j])
                s0 = int(starts[j])
                t = xpool.tile([128, 4, C], dt.float16, tag="xe", name="xe")
                (eng or nc.gpsimd).dma_start(
                    t[:],
                    xd.ap()[:, 4 * s0:4 * s0 + 4 * C].rearrange(
                        "p (c t) -> p c t", c=4
                    ),
                )
                xe_tiles[j] = t

            def expert(j):
                C = int(caps[j])
                s0 = int(starts[j])
                w13_sb = w13_tiles.pop(j)
                w2_sb = w2_tiles.pop(j)
                xe = xe_tiles.pop(j)

                he = wpool.tile([128, 2, C], dt.float16, tag="he", name="he")
                for hc in range(2):
                    ph1 = hpsum.tile([128, C], dt.float32, tag="ph", name="ph1")
                    for c in range(4):
                        nc.tensor.matmul(
                            ph1[:], lhsT=w13_sb[:, hc, c, :],
                            rhs=xe[:, c, :], start=(c == 0), stop=(c == 3),
                        )
                    ph3 = hpsum.tile([128, C], dt.float32, tag="ph", name="ph3")
                    for c in range(4):
                        nc.tensor.matmul(
                            ph3[:], lhsT=w13_sb[:, hc, 4 + c, :],
                            rhs=xe[:, c, :], start=(c == 0), stop=(c == 3),
                        )
                    t1 = wpool.tile([128, C], dt.float32, tag="t1", name="t1")
                    nc.scalar.activation(t1[:], ph1[:], AF.Silu)
                    nc.vector.tensor_tensor(
                        out=he[:, hc, :], in0=t1[:], in1=ph3[:], op=OP.mult
                    )

                yb = wpool.tile([128, 4, C], dt.float16, tag="yb", name="yb")
                for dc in range(4):
                    py = ypsum.tile([128, C], dt.float32, tag="py", name="py")
                    for hc in range(2):
                        nc.tensor.matmul(
                            py[:], lhsT=w2_sb[:, hc, ts(dc, 128)],
                            rhs=he[:, hc, :], start=(hc == 0), stop=(hc == 1),
                        )
                    # alternate engines so the copy stream keeps pace with L2
                    if dc % 2 == 0:
                        nc.scalar.copy(yb[:, dc, :], py[:])
                    else:
                        nc.vector.tensor_copy(yb[:, dc, :], py[:])
                # store issued on scalar right after the copies that fill yb:
                # its semaphores are already satisfied, so it never stalls the
                # scalar queue (and no loads live on this queue anyway).
                nc.scalar.dma_start(
                    yT.ap()[:, 4 * s0:4 * s0 + 4 * C].rearrange(
                        "p (c t) -> p c t", c=4
                    ),
                    yb[:],
                )

            def shared_group(g):
                hsh = wpool.tile([128, 4, GRP], dt.float16, tag="hsh", name="hsh")
                for hc in range(4):
                    ph1 = hpsum.tile([128, GRP], dt.float32, tag="ph", name="sph1")
                    for c in range(4):
                        nc.tensor.matmul(
                            ph1[:], lhsT=sw13_sb[:, c, ts(hc, 128)],
                            rhs=xs_sb[:, g, c, :], start=(c == 0), stop=(c == 3),
                        )
                    ph3 = hpsum.tile([128, GRP], dt.float32, tag="ph", name="sph3")
                    for c in range(4):
                        nc.tensor.matmul(
                            ph3[:], lhsT=sw13_sb[:, 4 + c, ts(hc, 128)],
                            rhs=xs_sb[:, g, c, :], start=(c == 0), stop=(c == 3),
                        )
                    t1 = wpool.tile([128, GRP], dt.float32, tag="t1", name="st1")
                    nc.scalar.activation(t1[:], ph1[:], AF.Silu)
                    nc.vector.tensor_tensor(
                        out=hsh[:, hc, :], in0=t1[:], in1=ph3[:], op=OP.mult
                    )
                yg = wpool.tile([128, 2, D], dt.float16, tag="yg", name="yg")
                for t2 in range(2):
                    py = ypsum.tile([128, D], dt.float32, tag="py", name="spy")
                    for hc in range(4):
                        nc.tensor.matmul(
                            py[:], lhsT=hsh[:, hc, ts(t2, 128)],
                            rhs=sw2_sb[:, hc, :], start=(hc == 0), stop=(hc == 3),
                        )
                    nc.vector.tensor_copy(yg[:, t2, :], py[:])
                nc.gpsimd.dma_start(
                    ysh.ap()[:, ts(g, 2 * D)].rearrange("p (c d) -> p c d", c=2),
                    yg[:],
                )

            # ---- preload all inputs (per-queue order = consumption order;
            # pool ring back-pressure bounds how far ahead each stream runs)
            # sync: weights in consumption order
            load_w13(0)
            load_w2(0)
            for j in (1, 2, 3, 4, 5, 6, 7):
                load_w13(j)
                load_w2(j)
            # E0 tokens ride the otherwise-loadless scalar queue: it issues
            # right after instruction load, parallel to sync's weight stream
            load_xe(0, eng=nc.scalar)
            load_sw13()
            load_xs(0)
            load_xs(1)
            load_sw2()
            load_xs(2)
            load_xs(3)
            for j in (1, 2, 3, 4, 5, 6, 7):
                load_xe(j)

            # ---- compute schedule, phase-separated: E0 covers the shared
            # consts' DMA, the shared phase covers the remaining experts'
            # input DMA, then the expert phase runs with a quiet bus so the
            # PE isn't slowed by concurrent DMA.
            for step in [0, "g0", "g1", "g2", "g3", 1, 2, 3, 4, 5, 6, 7]:
                if isinstance(step, str):
                    shared_group(int(step[1:]))
                else:
                    expert(step)

    nc.compile()
    return nc


# ---------------- host-side pack / combine ----------------

def _pshuf(a, nchunk):
    """[nchunk*128, F] -> [128, nchunk*F] partition-major contiguous."""
    f = a.shape[-1]
    return np.ascontiguousarray(
        a.reshape(nchunk, 128, f).transpose(1, 0, 2).reshape(128, nchunk * f)
    )


def host_prepare(xf, toks, assign, caps, w1, w3, w2, sw1, sw3, sw2):
    starts = np.concatenate([[0], np.cumsum(caps)]).astype(int)
    P = int(starts[-1])
    xfT16 = xf.T.astype(np.float16)                    # [D, N]
    x4 = xfT16.reshape(4, 128, N).transpose(1, 0, 2)   # [128, 4, N]
    w1h = np.asarray(w1, np.float32).astype(np.float16)
    w3h = np.asarray(w3, np.float32).astype(np.float16)
    w2h = np.asarray(w2, np.float32).astype(np.float16)
    # per-expert partition-major slabs, hc-major so halves load separately
    w13p = np.empty((E, 128, 2, 8, 128), np.float16)
    w13p[:, :, :, 0:4, :] = (
        w1h.reshape(E, 4, 128, 2, 128).transpose(0, 2, 3, 1, 4)
    )
    w13p[:, :, :, 4:8, :] = (
        w3h.reshape(E, 4, 128, 2, 128).transpose(0, 2, 3, 1, 4)
    )
    w13p = w13p.reshape(E, 128, 8 * H)
    w2p = w2h.reshape(E, 2, 128, D).transpose(0, 2, 1, 3).reshape(E, 128, 2 * D)
    sw13h = np.empty((128, 8 * HS), np.float16)
    sw13h[:, :4 * HS] = _pshuf(np.asarray(sw1, np.float32).astype(np.float16), 4)
    sw13h[:, 4 * HS:] = _pshuf(np.asarray(sw3, np.float32).astype(np.float16), 4)
    sw2ph = _pshuf(np.asarray(sw2, np.float32).astype(np.float16), 4)

    in_maps = []
    for c in range(NCORES):
        cols = np.zeros(P, np.int64)
        for j, e in enumerate(assign[c]):
            t = toks[e]
            cols[starts[j]:starts[j] + len(t)] = t
        xdc = x4[:, :, cols]                            # [128, 4, P]
        # segment-major flatten: expert j occupies cols [4*s_j, 4*s_j+4*C_j)
        xdp = np.empty((128, 4 * P), np.float16)
        for j in range(NSLOT):
            s0, C = int(starts[j]), int(caps[j])
            xdp[:, 4 * s0:4 * s0 + 4 * C] = xdc[:, :, s0:s0 + C].reshape(128, -1)
        xs = x4[:, :, c * NS:(c + 1) * NS]              # [128, 4, NS]
        xspc = np.empty((128, NGRP * 4 * GRP), np.float16)
        for g in range(NGRP):
            xspc[:, g * 4 * GRP:(g + 1) * 4 * GRP] = (
                xs[:, :, g * GRP:(g + 1) * GRP].reshape(128, -1)
            )
        in_maps.append({
            "xd": np.ascontiguousarray(xdp),
            "w13s": np.ascontiguousarray(w13p[assign[c]]),
            "w2s": np.ascontiguousarray(w2p[assign[c]]),
            "sw13": sw13h,
            "sw2p": sw2ph,
            "xsp": np.ascontiguousarray(xspc),
        })
    return in_maps, starts


def host_combine(res, toks, gvals, assign, starts):
    out = np.zeros((N, D), np.float32)
    for c, r in enumerate(res):
        yTc = np.asarray(r["yT"])                       # [128, 4*P]
        for j, e in enumerate(assign[c]):
            t = toks[e]
            n = len(t)
            if n == 0:
                continue
            s0 = int(starts[j])
            C = int(starts[j + 1]) - s0
            blk = yTc[:, 4 * s0:4 * s0 + 4 * C].reshape(128, 4, C)[:, :, :n]
            yseg = blk.transpose(2, 1, 0).reshape(n, D).astype(np.float32)
            out[t] += yseg * gvals[e][:, None]
        yshc = np.asarray(r["ysh"]).reshape(128, NGRP, 2, D)
        ysh_rows = yshc.transpose(1, 2, 0, 3).reshape(NS, D).astype(np.float32)
        out[c * NS:(c + 1) * NS] += ysh_rows
    return out.reshape(4, 2048, D)


_CACHE = {}


def kernel(x, gate_w, w1, w3, w2, sw1, sw3, sw2):
    from concourse.bass_utils import run_bass_kernel_spmd

    xf, toks, gvals, assign, caps = route_and_plan(x, gate_w)
    if caps not in _CACHE:
        _CACHE[caps] = build_kernel(caps)
    nc = _CACHE[caps]

    in_maps, starts = host_prepare(
        xf, toks, assign, caps, w1, w3, w2, sw1, sw3, sw2
    )
    res = run_bass_kernel_spmd(nc, in_maps, core_ids=list(range(NCORES))).results
    return host_combine(res, toks, gvals, assign, starts).astype(np.float32)


# revision 52
# speedup vs baseline: 1.0378x; 1.0378x over previous
"""Self-contained Trainium2 Bass kernel for nn_MoEWithDeepEP (8 NeuronCores).

Expert-parallel MoE, DeepEP-style split:
  host   - exact fp32 router (sigmoid top-2 + normalize), dispatch planning,
           token packing (the all-to-all bookkeeping), weighted combine.
  device - ONE fused kernel per core: 8 local experts' SwiGLU GEMMs over
           exactly-counted token segments (tokens ride the matmul free dim,
           so there is no capacity padding in compute) + the shared expert
           over this core's 1/8 token shard.

All device tensors are pre-shuffled on the host into partition-major
[128, F] contiguous layouts so every DMA is a dense 2D copy (few, large
descriptors). DMA issue is spread over four engine queues. The PE is
warmed with throwaway matmuls during the input DMA lead-in so real work
runs at full clock.

The per-slot segment capacities depend on the routing realized by the
inputs, so the device program is built (and cached) per capacity tuple.
"""
import sys
for _p in ("/opt/trn_rl_repo", "/root/.axon_site/_ro/trn_rl_repo"):
    if _p not in sys.path:
        sys.path.insert(0, _p)

import numpy as np

N = 8192          # tokens
D = 512           # model dim
E = 64            # experts
K = 2             # top-k
H = 256           # expert hidden
HS = 512          # shared hidden (H * num_shared)
NCORES = 8
NSLOT = E // NCORES   # 8 expert slots per core
NS = N // NCORES      # shared-expert tokens per core
GRP = 256             # shared-expert token group
NGRP = NS // GRP
ROUTE_SCALE = 2.5
WARM_MM = 32          # PE warmup matmuls during DMA lead-in


def _mk_bacc():
    from concourse import bacc

    return bacc.Bacc(
        "TRN2",
        target_bir_lowering=False,
        debug=False,
        enable_asserts=False,
        num_devices=NCORES,
    )


# ---------------- host-side routing / planning ----------------

def route_and_plan(x, gate_w):
    """Exact fp32 router + expert->core assignment + slot capacities."""
    xf = np.ascontiguousarray(np.asarray(x, np.float32).reshape(N, D))
    logits = xf @ np.asarray(gate_w, np.float32).T
    scores = 1.0 / (1.0 + np.exp(-logits))
    top_idx = np.argsort(-scores, axis=1, kind="stable")[:, :K]
    tsc = np.take_along_axis(scores, top_idx, 1)
    gat = (tsc / (tsc.sum(1, keepdims=True) + 1e-20) * ROUTE_SCALE).astype(
        np.float32
    )
    counts = np.bincount(top_idx.ravel(), minlength=E)

    toks, gvals = [], []
    for e in range(E):
        t, kk = np.nonzero(top_idx == e)
        toks.append(t)
        gvals.append(gat[t, kk])

    # LPT: exactly NSLOT experts per core, balancing token load
    order = np.argsort(-counts, kind="stable")
    loads = [0] * NCORES
    assign = [[] for _ in range(NCORES)]
    for e in order:
        c = min(
            (c for c in range(NCORES) if len(assign[c]) < NSLOT),
            key=lambda c: loads[c],
        )
        assign[c].append(int(e))
        loads[c] += int(counts[e])
    for c in range(NCORES):
        assign[c].sort(key=lambda e: -counts[e])

    # slot capacities: rank-wise max across cores, 8-aligned
    caps = tuple(
        max(8, int(-(-max(counts[assign[c][j]] for c in range(NCORES)) // 8) * 8))
        for j in range(NSLOT)
    )
    return xf, toks, gvals, assign, caps


# ---------------- device kernel ----------------

def build_kernel(caps):
    import concourse.bass as bass
    import concourse.tile as tile
    from concourse import mybir

    dt = mybir.dt
    AF = mybir.ActivationFunctionType
    OP = mybir.AluOpType
    ts = bass.ts
    nc = _mk_bacc()

    starts = np.concatenate([[0], np.cumsum(caps)]).astype(int)
    P = int(starts[-1])

    # all inputs/outputs pre-shuffled to partition-major [128, F] layouts
    xd = nc.dram_tensor("xd", [128, 4 * P], dt.float16, kind="ExternalInput")
    w13s = nc.dram_tensor("w13s", [NSLOT, 128, 8 * H], dt.float16,
                          kind="ExternalInput")
    w2s = nc.dram_tensor("w2s", [NSLOT, 128, 2 * D], dt.float16,
                         kind="ExternalInput")
    sw13 = nc.dram_tensor("sw13", [128, 8 * HS], dt.float16, kind="ExternalInput")
    sw2p = nc.dram_tensor("sw2p", [128, 4 * D], dt.float16, kind="ExternalInput")
    xsp = nc.dram_tensor("xsp", [128, NGRP * 4 * GRP], dt.float16,
                         kind="ExternalInput")

    yT = nc.dram_tensor("yT", [128, 4 * P], dt.float16, kind="ExternalOutput")
    ysh = nc.dram_tensor("ysh", [128, NGRP * 2 * D], dt.float16,
                         kind="ExternalOutput")

    with tile.TileContext(nc) as tc:
        with (
            tc.tile_pool(name="const", bufs=1) as cpool,
            tc.tile_pool(name="ew", bufs=8) as ewpool,
            tc.tile_pool(name="xe", bufs=8) as xpool,
            tc.tile_pool(name="work", bufs=3) as wpool,
            tc.tile_pool(name="hps", bufs=4, space="PSUM") as hpsum,
            tc.tile_pool(name="yps", bufs=4, space="PSUM") as ypsum,
        ):
            # ---- PE warmup: throwaway matmuls with no input deps ----
            wsrc = cpool.tile([128, 128], dt.float16, name="wsrc")
            nc.vector.memset(wsrc[:], 0)
            for _ in range(WARM_MM):
                wps = ypsum.tile([128, 128], dt.float32, tag="py", name="wps")
                nc.tensor.matmul(
                    wps[:], lhsT=wsrc[:], rhs=wsrc[:, 0:1].to_broadcast([128, 128]),
                    start=True, stop=True, skip_group_check=True,
                )

            sw13_sb = cpool.tile([128, 8, HS], dt.float16, name="sw13_sb")
            sw2_sb = cpool.tile([128, 4, D], dt.float16, name="sw2_sb")
            xs_sb = cpool.tile([128, NGRP, 4, GRP], dt.float16, name="xs_sb")

            # ---- preload: pure-load queues, no store ever blocks a load.
            # sync carries weights, gpsimd carries activations; pool ring
            # back-pressure paces the prefetch depth automatically.
            w13_tiles, w2_tiles, xe_tiles = {}, {}, {}

            def load_w13(j):
                # hc-major halves: the first 0.26MB half unblocks L1 hc=0
                t = ewpool.tile([128, 2, 8, 128], dt.float16, tag="w13",
                                name="w13_sb")
                for hc in range(2):
                    nc.sync.dma_start(
                        t[:, hc],
                        w13s.ap()[j][:, ts(hc, 8 * 128)].rearrange(
                            "p (c h) -> p c h", c=8
                        ),
                    )
                w13_tiles[j] = t

            def load_w2(j):
                t = ewpool.tile([128, 2, D], dt.float16, tag="w2", name="w2_sb")
                nc.sync.dma_start(
                    t[:], w2s.ap()[j].rearrange("p (c d) -> p c d", c=2)
                )
                w2_tiles[j] = t

            def load_sw13():
                nc.gpsimd.dma_start(
                    sw13_sb[:], sw13.ap().rearrange("p (c h) -> p c h", c=8)
                )

            def load_sw2():
                nc.gpsimd.dma_start(
                    sw2_sb[:], sw2p.ap().rearrange("p (c d) -> p c d", c=4)
                )

            def load_xs(g):
                nc.gpsimd.dma_start(
                    xs_sb[:, g],
                    xsp.ap()[:, ts(g, 4 * GRP)].rearrange(
                        "p (c t) -> p c t", c=4
                    ),
                )

            def load_xe(j):
                C = int(caps[j])
                s0 = int(starts[j])
                t = xpool.tile([128, 4, C], dt.float16, tag="xe", name="xe")
                nc.gpsimd.dma_start(
                    t[:],
                    xd.ap()[:, 4 * s0:4 * s0 + 4 * C].rearrange(
                        "p (c t) -> p c t", c=4
                    ),
                )
                xe_tiles[j] = t

            def expert(j):
                C = int(caps[j])
                s0 = int(starts[j])
                w13_sb = w13_tiles.pop(j)
                w2_sb = w2_tiles.pop(j)
                xe = xe_tiles.pop(j)

                he = wpool.tile([128, 2, C], dt.float16, tag="he", name="he")
                for hc in range(2):
                    ph1 = hpsum.tile([128, C], dt.float32, tag="ph", name="ph1")
                    for c in range(4):
                        nc.tensor.matmul(
                            ph1[:], lhsT=w13_sb[:, hc, c, :],
                            rhs=xe[:, c, :], start=(c == 0), stop=(c == 3),
                        )
                    ph3 = hpsum.tile([128, C], dt.float32, tag="ph", name="ph3")
                    for c in range(4):
                        nc.tensor.matmul(
                            ph3[:], lhsT=w13_sb[:, hc, 4 + c, :],
                            rhs=xe[:, c, :], start=(c == 0), stop=(c == 3),
                        )
                    t1 = wpool.tile([128, C], dt.float32, tag="t1", name="t1")
                    nc.scalar.activation(t1[:], ph1[:], AF.Silu)
                    nc.vector.tensor_tensor(
                        out=he[:, hc, :], in0=t1[:], in1=ph3[:], op=OP.mult
                    )

                yb = wpool.tile([128, 4, C], dt.float16, tag="yb", name="yb")
                for dc in range(4):
                    py = ypsum.tile([128, C], dt.float32, tag="py", name="py")
                    for hc in range(2):
                        nc.tensor.matmul(
                            py[:], lhsT=w2_sb[:, hc, ts(dc, 128)],
                            rhs=he[:, hc, :], start=(hc == 0), stop=(hc == 1),
                        )
                    # alternate engines so the copy stream keeps pace with L2
                    if dc % 2 == 0:
                        nc.scalar.copy(yb[:, dc, :], py[:])
                    else:
                        nc.vector.tensor_copy(yb[:, dc, :], py[:])
                # store issued on scalar right after the copies that fill yb:
                # its semaphores are already satisfied, so it never stalls the
                # scalar queue (and no loads live on this queue anyway).
                nc.scalar.dma_start(
                    yT.ap()[:, 4 * s0:4 * s0 + 4 * C].rearrange(
                        "p (c t) -> p c t", c=4
                    ),
                    yb[:],
                )

            def shared_group(g):
                hsh = wpool.tile([128, 4, GRP], dt.float16, tag="hsh", name="hsh")
                for hc in range(4):
                    ph1 = hpsum.tile([128, GRP], dt.float32, tag="ph", name="sph1")
                    for c in range(4):
                        nc.tensor.matmul(
                            ph1[:], lhsT=sw13_sb[:, c, ts(hc, 128)],
                            rhs=xs_sb[:, g, c, :], start=(c == 0), stop=(c == 3),
                        )
                    ph3 = hpsum.tile([128, GRP], dt.float32, tag="ph", name="sph3")
                    for c in range(4):
                        nc.tensor.matmul(
                            ph3[:], lhsT=sw13_sb[:, 4 + c, ts(hc, 128)],
                            rhs=xs_sb[:, g, c, :], start=(c == 0), stop=(c == 3),
                        )
                    t1 = wpool.tile([128, GRP], dt.float32, tag="t1", name="st1")
                    nc.scalar.activation(t1[:], ph1[:], AF.Silu)
                    nc.vector.tensor_tensor(
                        out=hsh[:, hc, :], in0=t1[:], in1=ph3[:], op=OP.mult
                    )
                yg = wpool.tile([128, 2, D], dt.float16, tag="yg", name="yg")
                for t2 in range(2):
                    py = ypsum.tile([128, D], dt.float32, tag="py", name="spy")
                    for hc in range(4):
                        nc.tensor.matmul(
                            py[:], lhsT=hsh[:, hc, ts(t2, 128)],
                            rhs=sw2_sb[:, hc, :], start=(hc == 0), stop=(hc == 3),
                        )
                    nc.vector.tensor_copy(yg[:, t2, :], py[:])
                nc.gpsimd.dma_start(
                    ysh.ap()[:, ts(g, 2 * D)].rearrange("p (c d) -> p c d", c=2),
                    yg[:],
                )

            # ---- preload all inputs (per-queue order = consumption order;
            # pool ring back-pressure bounds how far ahead each stream runs)
            # sync: weights in consumption order
            load_w13(0)
            load_w2(0)
            for j in (1, 2, 3, 4, 5, 6, 7):
                load_w13(j)
                load_w2(j)
            # gpsimd: E0 tokens, then shared consts, then remaining tokens
            load_xe(0)
            load_sw13()
            load_xs(0)
            load_xs(1)
            load_sw2()
            load_xs(2)
            load_xs(3)
            for j in (1, 2, 3, 4, 5, 6, 7):
                load_xe(j)

            # ---- compute schedule, phase-separated: E0 covers the shared
            # consts' DMA, the shared phase covers the remaining experts'
            # input DMA, then the expert phase runs with a quiet bus so the
            # PE isn't slowed by concurrent DMA.
            for step in [0, "g0", "g1", "g2", "g3", 1, 2, 3, 4, 5, 6, 7]:
                if isinstance(step, str):
                    shared_group(int(step[1:]))
                else:
                    expert(step)

    nc.compile()
    return nc


# ---------------- host-side pack / combine ----------------

def _pshuf(a, nchunk):
    """[nchunk*128, F] -> [128, nchunk*F] partition-major contiguous."""
    f = a.shape[-1]
    return np.ascontiguousarray(
        a.reshape(nchunk, 128, f).transpose(1, 0, 2).reshape(128, nchunk * f)
    )


def host_prepare(xf, toks, assign, caps, w1, w3, w2, sw1, sw3, sw2):
    starts = np.concatenate([[0], np.cumsum(caps)]).astype(int)
    P = int(starts[-1])
    xfT16 = xf.T.astype(np.float16)                    # [D, N]
    x4 = xfT16.reshape(4, 128, N).transpose(1, 0, 2)   # [128, 4, N]
    w1h = np.asarray(w1, np.float32).astype(np.float16)
    w3h = np.asarray(w3, np.float32).astype(np.float16)
    w2h = np.asarray(w2, np.float32).astype(np.float16)
    # per-expert partition-major slabs, hc-major so halves load separately
    w13p = np.empty((E, 128, 2, 8, 128), np.float16)
    w13p[:, :, :, 0:4, :] = (
        w1h.reshape(E, 4, 128, 2, 128).transpose(0, 2, 3, 1, 4)
    )
    w13p[:, :, :, 4:8, :] = (
        w3h.reshape(E, 4, 128, 2, 128).transpose(0, 2, 3, 1, 4)
    )
    w13p = w13p.reshape(E, 128, 8 * H)
    w2p = w2h.reshape(E, 2, 128, D).transpose(0, 2, 1, 3).reshape(E, 128, 2 * D)
    sw13h = np.empty((128, 8 * HS), np.float16)
    sw13h[:, :4 * HS] = _pshuf(np.asarray(sw1, np.float32).astype(np.float16), 4)
    sw13h[:, 4 * HS:] = _pshuf(np.asarray(sw3, np.float32).astype(np.float16), 4)
    sw2ph = _pshuf(np.asarray(sw2, np.float32).astype(np.float16), 4)

    in_maps = []
    for c in range(NCORES):
        cols = np.zeros(P, np.int64)
        for j, e in enumerate(assign[c]):
            t = toks[e]
            cols[starts[j]:starts[j] + len(t)] = t
        xdc = x4[:, :, cols]                            # [128, 4, P]
        # segment-major flatten: expert j occupies cols [4*s_j, 4*s_j+4*C_j)
        xdp = np.empty((128, 4 * P), np.float16)
        for j in range(NSLOT):
            s0, C = int(starts[j]), int(caps[j])
            xdp[:, 4 * s0:4 * s0 + 4 * C] = xdc[:, :, s0:s0 + C].reshape(128, -1)
        xs = x4[:, :, c * NS:(c + 1) * NS]              # [128, 4, NS]
        xspc = np.empty((128, NGRP * 4 * GRP), np.float16)
        for g in range(NGRP):
            xspc[:, g * 4 * GRP:(g + 1) * 4 * GRP] = (
                xs[:, :, g * GRP:(g + 1) * GRP].reshape(128, -1)
            )
        in_maps.append({
            "xd": np.ascontiguousarray(xdp),
            "w13s": np.ascontiguousarray(w13p[assign[c]]),
            "w2s": np.ascontiguousarray(w2p[assign[c]]),
            "sw13": sw13h,
            "sw2p": sw2ph,
            "xsp": np.ascontiguousarray(xspc),
        })
    return in_maps, starts


def host_combine(res, toks, gvals, assign, starts):
    out = np.zeros((N, D), np.float32)
    for c, r in enumerate(res):
        yTc = np.asarray(r["yT"])                       # [128, 4*P]
        for j, e in enumerate(assign[c]):
            t = toks[e]
            n = len(t)
            if n == 0:
                continue
            s0 = int(starts[j])
            C = int(starts[j + 1]) - s0
            blk = yTc[:, 4 * s0:4 * s0 + 4 * C].reshape(128, 4, C)[:, :, :n]
            yseg = blk.transpose(2, 1, 0).reshape(n, D).astype(np.float32)
            out[t] += yseg * gvals[e][:, None]
        yshc = np.asarray(r["ysh"]).reshape(128, NGRP, 2, D)
        ysh_rows = yshc.transpose(1, 2, 0, 3).reshape(NS, D).astype(np.float32)
        out[c * NS:(c + 1) * NS] += ysh_rows
    return out.reshape(4, 2048, D)


_CACHE = {}


def kernel(x, gate_w, w1, w3, w2, sw1, sw3, sw2):
    from concourse.bass_utils import run_bass_kernel_spmd

    xf, toks, gvals, assign, caps = route_and_plan(x, gate_w)
    if caps not in _CACHE:
        _CACHE[caps] = build_kernel(caps)
    nc = _CACHE[caps]

    in_maps, starts = host_prepare(
        xf, toks, assign, caps, w1, w3, w2, sw1, sw3, sw2
    )
    res = run_bass_kernel_spmd(nc, in_maps, core_ids=list(range(NCORES))).results
    return host_combine(res, toks, gvals, assign, starts).astype(np.float32)
